# revision 12
# baseline (speedup 1.0000x reference)
"""CRF head kernel for Trainium2 (Bass/Tile), 8-core data-parallel, v2.

Computes: out[b, t, :] = x[b, t, :] + transitions[argmax(x[b, t, :]), :]
for x of shape [128, 1024, 256] f32 and transitions [256, 256] f32.

v2 design (vs v1's SWDGE HBM gather + index DRAM round-trip):
  * The gather is done ON-CHIP by the PE array: a bf16 one-hot of the
    row max (is_equal against the row max) is PE-transposed per
    128-column chunk, then matmul'd with a bf16 copy of `transitions`
    accumulated in PSUM: out_psum[row, :] = sum_k onehot[row, k] * T[k, :]
    = T[argmax(row), :]. No HBM gather traffic, no per-row descriptors.
  * Batch split across 8 cores (16384 rows/core). Megatile = 4096 rows
    laid out [128 partitions x 32 rows x 256 tags] f32 (4 MB), so every
    HBM load descriptor is 32 KB contiguous per partition.
  * Output is stored as bf16 (halves store traffic); host upcasts to
    f32. Tolerance budget: |T| bf16 err <= ~0.02 + bf16 out round
    <= ~0.03 << 2e-2 * scale (~0.145).
  * Engine split per megatile: DVE reduce_max + final adds; GpSimd
    is_equal (one-hot); PE 64 transposes + 64 matmuls; Act (scalar)
    PSUM->SBUF one-hot-T copies + y stores; SP (sync) x loads.
  * Rows whose f32 max is attained at >1 tag (exact ties; argmax must
    pick the first) are corrected exactly on the host — the one-hot
    would sum both transition rows. ~3 rows per 131072 in practice.
"""

import sys

for _p in ("/opt/trn_rl_repo",):
    if _p not in sys.path:
        sys.path.append(_p)

import numpy as np
import ml_dtypes

import concourse.bass as bass
import concourse.bacc as bacc
import concourse.mybir as mybir
import concourse.tile as tile
import concourse.bass_utils as bass_utils
from concourse.masks import make_identity

N_CORES = 8
B, T, TAGS = 128, 1024, 256
R = (B // N_CORES) * T          # rows per core = 16384
P = 128                         # SBUF partitions
G = 16                          # rows per partition per megatile
ROWS_PER_MT = P * G             # 4096
M = R // ROWS_PER_MT            # 4 megatiles per core
GRP = 4                         # g's per PSUM group
NGRP = G // GRP                 # groups per megatile = 8

F32 = mybir.dt.float32
BF16 = mybir.dt.bfloat16

_CACHE = {}


def _build():
    nc = bacc.Bacc("TRN2", target_bir_lowering=False, debug=False)

    x = nc.dram_tensor("x", [R, TAGS], F32, kind="ExternalInput")
    t = nc.dram_tensor("t", [TAGS, TAGS], BF16, kind="ExternalInput")
    y = nc.dram_tensor("y", [R, TAGS], BF16, kind="ExternalOutput")

    # megatile m, partition p holds rows m*4096 + p*G .. +G-1 (contiguous)
    xv = x.ap().rearrange("(m p g) d -> m p (g d)", p=P, g=G)
    yv = y.ap().rearrange("(m p g) d -> m p (g d)", p=P, g=G)

    with tile.TileContext(nc) as tc:
        with (
            tc.tile_pool(name="cp", bufs=1) as cp,
            tc.tile_pool(name="xp", bufs=4) as xp,
            tc.tile_pool(name="op", bufs=3) as op,
            tc.tile_pool(name="sp", bufs=2) as sp,
            tc.tile_pool(name="tp", bufs=4) as tp,
            tc.tile_pool(name="bp", bufs=4) as bp,
            tc.tile_pool(name="yp", bufs=3) as yp,
            tc.tile_pool(name="pp", bufs=3, space="PSUM") as pp,
            tc.tile_pool(name="qp", bufs=2, space="PSUM") as qp,
        ):
            ident = cp.tile([P, P], BF16, tag="ident", name="ident")
            make_identity(nc, ident)
            # tr[p, c, :] = transitions[c*128 + p, :] as bf16
            tr = cp.tile([P, 2, TAGS], BF16, tag="tr", name="tr")
            nc.sync.dma_start(
                out=tr[:], in_=t.ap().rearrange("(c p) n -> p c n", p=P))

            for m in range(M):
                x_t = xp.tile([P, G * TAGS], F32, tag="x", name=f"x_{m}")
                nc.sync.dma_start(out=x_t[:], in_=xv[m])
                x3 = x_t[:].rearrange("p (g d) -> p g d", d=TAGS)

                mx = sp.tile([P, G], F32, tag="mx", name=f"mx_{m}")
                nc.vector.tensor_reduce(out=mx[:], in_=x3,
                                        axis=mybir.AxisListType.X,
                                        op=mybir.AluOpType.max)

                oh = op.tile([P, G * TAGS], BF16, tag="oh", name=f"oh_{m}")
                oh3 = oh[:].rearrange("p (g d) -> p g d", d=TAGS)
                mx_b = mx[:].unsqueeze(2).broadcast_to([P, G, TAGS])
                nc.vector.tensor_tensor(out=oh3, in0=x3, in1=mx_b,
                                        op=mybir.AluOpType.is_equal)

                # f32 y tile: DVE adds run 2 elem/cyc with f32 output; the
                # SWDGE store casts f32->bf16 in flight (Pool-only feature).
                y_t = yp.tile([P, G * TAGS], F32, tag="y", name=f"y_{m}")

                for grp in range(NGRP):
                    # 8 transpose blocks (4 g's x 2 tag chunks) -> one bank
                    ohT_ps = pp.tile([P, GRP * 2 * P], BF16, tag="ohT_ps",
                                     name=f"ohT_ps_{m}_{grp}")
                    for j in range(GRP * 2):
                        g = grp * GRP + j // 2
                        c = j % 2
                        nc.tensor.transpose(
                            ohT_ps[:, j * P:(j + 1) * P],
                            oh[:, g * TAGS + c * P: g * TAGS + (c + 1) * P],
                            ident[:],
                        )
                    ohT_sb = tp.tile([P, GRP * 2 * P], BF16, tag="ohT_sb",
                                     name=f"ohT_sb_{m}_{grp}")
                    nc.scalar.copy(out=ohT_sb[:], in_=ohT_ps[:])

                    lo = grp * GRP * TAGS
                    hi = (grp + 1) * GRP * TAGS
                    out_ps = qp.tile([P, GRP * TAGS], F32, tag="out_ps",
                                     name=f"out_ps_{m}_{grp}")
                    for jg in range(GRP):
                        sl = out_ps[:, jg * TAGS:(jg + 1) * TAGS]
                        nc.tensor.matmul(
                            sl, ohT_sb[:, (2 * jg) * P:(2 * jg + 1) * P],
                            tr[:, 0, :], start=True, stop=False)
                        nc.tensor.matmul(
                            sl, ohT_sb[:, (2 * jg + 1) * P:(2 * jg + 2) * P],
                            tr[:, 1, :], start=False, stop=True)

                    # Act drains PSUM (full-rate psum read); DVE then adds
                    # SBUF+SBUF at 2 elem/cycle instead of 1 on psum.
                    gath = bp.tile([P, GRP * TAGS], F32, tag="gath",
                                   name=f"gath_{m}_{grp}")
                    nc.scalar.copy(out=gath[:], in_=out_ps[:])
                    nc.vector.tensor_tensor(
                        out=y_t[:, lo:hi], in0=x_t[:, lo:hi], in1=gath[:],
                        op=mybir.AluOpType.add)

                nc.gpsimd.dma_start(out=yv[m], in_=y_t[:])

    nc.compile()
    return nc


def get_nc():
    if "nc" not in _CACHE:
        _CACHE["nc"] = _build()
    return _CACHE["nc"]


def kernel(launch_matrix, transitions):
    launch = np.ascontiguousarray(np.asarray(launch_matrix, dtype=np.float32))
    trans = np.ascontiguousarray(np.asarray(transitions, dtype=np.float32))
    assert launch.shape == (B, T, TAGS), launch.shape
    assert trans.shape == (TAGS, TAGS), trans.shape

    nc = get_nc()
    shards = launch.reshape(N_CORES, R, TAGS)
    trans_bf16 = trans.astype(ml_dtypes.bfloat16)
    in_maps = [{"x": shards[c], "t": trans_bf16} for c in range(N_CORES)]
    res = bass_utils.run_bass_kernel_spmd(nc, in_maps,
                                          core_ids=list(range(N_CORES)))
    _CACHE["last_results"] = res
    out = np.concatenate(
        [np.asarray(res.results[c]["y"]) for c in range(N_CORES)], axis=0)
    out = out.astype(np.float32).reshape(B, T, TAGS)

    # Exact-tie correction: the device one-hot sums transition rows for
    # every tag attaining the row max; argmax semantics require the first.
    flat = launch.reshape(-1, TAGS)
    mx = flat.max(axis=1, keepdims=True)
    eq = flat == mx
    tie_rows = np.flatnonzero(eq.sum(axis=1) > 1)
    if tie_rows.size:
        first = eq[tie_rows].argmax(axis=1)
        out.reshape(-1, TAGS)[tie_rows] = flat[tie_rows] + trans[first]
    return out
